# revision 44
# baseline (speedup 1.0000x reference)
"""Trainium2 Bass kernel for nn_Meta_67078799229377 (relation-network meta-learner).

Sharding: 8 cores = 4 batch elements x 2 halves of the relation-j axis.
Each core runs the full backbone for its batch element's 6 images, then the
relation g-MLP for its 18 (i, j) pairs fully fused on-chip.  The device only
produces (a) per-image channel sums `fme` for the cls head and (b) the
(q,p)-summed relation features `xf`; the tiny f/cls MLP heads and loss
reductions run on the host in f64.

Engine plan (measured rates):
  - hdd = relu(v + u_q): DVE tensor_scalar, ~229ns i2i per [128,384]
    (2x perf mode; 1-op and 2-op variants measure identically, so the
    max(v,-u) factorization buys nothing).  A few units go to ACT
    (505ns each) to absorb its idle head.  This stream is the relation
    bottleneck: 178 ops x 229ns = 40.8us, LP-tight against ACT's gscr.
  - g matmuls: PE pairs at tile_position (0,0)/(0,64).  The PE runs at
    its mid p-state (~427ns/pair) throughout the relation phase: the
    full 2.4GHz state needs ~8us of unbroken execution which the
    psum-rotation stalls (drain 2.18us > fill 1.7us) never allow.
    That's fine steady-state (ACT-paced) and costs ~2.5us in the tail.
  - gscr relu+bias+sum: ACT activation w/ accum_out, 2.18us per
    [128,2048] tile incl the 283ns ACTIVATION_READ_ACCUMULATOR.
    The final two tiles drain split ACT || DVE halves (KSPLIT).
  - Input DMAs ride both HWDGE queues (sync + ACT) concurrently;
    conv1 start is bound by DMA completion latency (~10.5us).
  - gpsimd: memsets only (tensor ops crash this ucode build).
  - Teardown (~9.3us: 325 sem-reset/barrier instrs) is framework-fixed.
"""
import os
import numpy as np
import ml_dtypes

import concourse.bass as bass
import concourse.mybir as mybir
import concourse.tile as tile
from concourse import bacc
from concourse.bass_utils import run_bass_kernel_spmd

F32 = mybir.dt.float32
BF16 = mybir.dt.bfloat16
AF = mybir.ActivationFunctionType
OP = mybir.AluOpType

B, S, D = 4, 6, 8
M = D * D            # 64 spatial positions
C2 = 66              # 64 channels + 2 coord channels
H1 = 128             # g-MLP hidden
CO = 64              # g-MLP out
NCls = 64
N_CORES = 8

# bf16 const blob column layout
CB_W1 = 0            # [27, 32]
CB_W2 = 32           # [32, 432]
CB_W3 = 464          # [48, 576]
CB_W1A = 1040        # [66, 128]
CB_W1B = 1168        # [66, 128]
CB_WG2 = 1296        # [128, 64]
CB_COORD = 1360      # [2, 384]
CB_N = 1744

# f32 const blob column layout: bc1, bc2, bc3, bg1, bg2(x2)
CF_N = 5

KWARM = int(os.environ.get("KWARM", "5"))
KH_ACT = int(os.environ.get("KH_ACT", "1"))   # hdd instrs per unit on ACT
KG_V = int(os.environ.get("KG_V", "0"))       # gscr instrs per unit on DVE
KFILL = int(os.environ.get("KFILL", "2"))     # PE filler pairs per conv gap
# tail gscr tiles handled by DVE as "unit:duo" pairs (DVE is idle after the
# last hdd, ACT otherwise serializes the last unit's three tiles)
KDVE = {tuple(int(x) for x in ud.split(":"))
        for ud in os.environ.get("KDVE", "").split(",") if ud}
KSPLIT = os.environ.get("KSPLIT", "1") == "1"  # split the final tile's drain


def _build_nc():
    nc = bacc.Bacc("TRN2", target_bir_lowering=False, debug=False,
                   num_devices=N_CORES)

    x_pk = nc.dram_tensor("pk", [128, 4, 512], BF16, kind="ExternalInput")
    x_cb = nc.dram_tensor("cb", [128, CB_N], BF16, kind="ExternalInput")
    x_cf = nc.dram_tensor("cf", [128, CF_N], F32, kind="ExternalInput")

    out_fme = nc.dram_tensor("fme", [64, S], F32, kind="ExternalOutput")
    out_xf = nc.dram_tensor("xf", [128, 38], F32, kind="ExternalOutput")

    with tile.TileContext(nc) as tc:
        with (
            tc.tile_pool(name="const", bufs=1) as cpool,
            tc.tile_pool(name="work", bufs=1) as wpool,
            tc.tile_pool(name="patch", bufs=1) as ppool,
            tc.tile_pool(name="hdd", bufs=int(os.environ.get("KHB", "3"))) as hpool,
            tc.tile_pool(name="gscr", bufs=3) as spool,
        ):
            _stages = ["c1", "c2", "c3", "uv", "rel", "full"]
            _stop = os.environ.get("KSTOP", "full")
            def _do(stage):
                return _stages.index(stage) <= _stages.index(_stop)

            # ---- scratch + warmup (no input deps: runs during DMA) ----
            wsrc = cpool.tile([128, 512], BF16, tag="wsrc")
            nc.gpsimd.memset(wsrc[:], 0.0)
            ttrig = wpool.tile([128, 2], F32, tag="ttrig")

            cb = cpool.tile([128, CB_N], BF16, tag="cb")
            cf = cpool.tile([128, CF_N], F32, tag="cf")
            pk_sb = ppool.tile([128, 4, 512], BF16)
            # DMA order follows the consumption order: conv1 patches first
            # (conv1 is the head of the dependency chain), then conv weights,
            # then relation weights/biases.  Slices match contiguity in DRAM
            # (2KB+ per-partition chunks) to keep DMA at full rate.
            # two HWDGE queues: patches on the ACT queue run concurrently
            # with weights on the sync queue
            nc.scalar.dma_start(out=pk_sb[:, 0:2, :], in_=x_pk[:, 0:2, :])
            nc.sync.dma_start(out=cb[:, 0:CB_W1A], in_=x_cb[:, 0:CB_W1A])
            nc.scalar.dma_start(out=pk_sb[:, 2:4, :], in_=x_pk[:, 2:4, :])
            nc.sync.dma_start(out=cb[:, CB_W1A:], in_=x_cb[:, CB_W1A:])
            nc.sync.dma_start(out=cf[:], in_=x_cf[:])

            # trigger the ACT function-table load early (relu set)
            nc.scalar.activation(ttrig[:], wsrc[:, 0:2], AF.Relu)

            w1a = cb[0:C2, CB_W1A:CB_W1A + H1]
            w1b = cb[0:C2, CB_W1B:CB_W1B + H1]
            wg2 = cb[:, CB_WG2:CB_WG2 + CO]
            bc1 = cf[:, 0:1]        # replicated x4 partition groups
            bc2 = cf[0:112, 1:2]    # replicated at parts 0:48 and 64:112
            bc3 = cf[0:64, 2:3]
            bg1 = cf[:, 3:4]
            bg2 = cf[:, 4:5]

            featc = wpool.tile([C2, S * M], BF16)
            nc.vector.tensor_copy(featc[64:66, :], cb[0:2, CB_COORD:CB_COORD + 384])

            xf_all = wpool.tile([128, 38], F32, tag="xfall")
            nc.gpsimd.memset(xf_all[:], 0.0)
            xf_a = xf_all[:, 0:18]
            xf_d = xf_all[:, 20:38]

            zb2048 = cpool.tile([128, 2048], BF16, tag="zb")
            nc.gpsimd.memset(zb2048[:], 0.0)

            # c1sb[32*(i%4)+c, i//4, y, x] holds conv1 output of image i
            # c2sb[64*(i%2)+c, i//2, y, x] holds conv2 output of image i
            c1sb = wpool.tile([128, 2, 33, 33], BF16)
            c2sb = wpool.tile([112, 3, 17, 17], BF16)
            for d in range(2):
                nc.gpsimd.memset(c1sb[:, d, 32, :], 0.0)
                nc.gpsimd.memset(c1sb[:, d, 0:32, 32], 0.0)
            for d in range(3):
                nc.gpsimd.memset(c2sb[:, d, 16, :], 0.0)
                nc.gpsimd.memset(c2sb[:, d, 0:16, 16], 0.0)

            with (
                tc.tile_pool(name="pconv", bufs=2, space="PSUM") as pc_pool,
                tc.tile_pool(name="psmall", bufs=2, space="PSUM") as ps_pool,
                tc.tile_pool(name="pcb", bufs=1, space="PSUM") as pcb_pool,
                tc.tile_pool(name="pwarm", bufs=1, space="PSUM") as pw_pool,
            ):
                psw = pw_pool.tile([128, 512], F32, tag="warm")

                def _pe_fill(n):
                    # dependency-free matmuls that keep the PE clock ramped
                    # through relu-wait gaps between conv stages
                    for r in range(n):
                        nc.tensor.matmul(psw[0:64, :], wsrc[:, 0:64], wsrc[:],
                                         start=True, stop=True,
                                         tile_position=(0, 0))
                        nc.tensor.matmul(psw[64:128, :], wsrc[:, 0:64],
                                         wsrc[:],
                                         start=True, stop=True,
                                         tile_position=(0, 64))

                _pe_fill(KWARM)

                # ---- conv1: [27]->[32], 4 images per matmul round via
                # tile_position packing (diagonal 32x32 tiles).  Both d-halves
                # of each y-half share one psum tile so the epilogue is two
                # big concurrent relus (ACT || DVE) instead of four.
                # pA/pB rows 64:128 at d=1 are never written; the matching
                # c1sb region is unused, so streaming garbage there is fine.
                ps1h = [pc_pool.tile([128, 2, 16, 32], F32, tag="psc",
                                     name=f"ps1h{h}") for h in range(2)]
                for blk in range(4):
                    groups = (0, 1, 2, 3) if blk < 2 else (0, 1)
                    h, d = blk % 2, blk // 2
                    for g in groups:
                        nc.tensor.matmul(
                            ps1h[h][32 * g:32 * g + 32, d, :, :].rearrange(
                                "p a b -> p (a b)"),
                            cb[32 * g:32 * g + 27, CB_W1:CB_W1 + 32],
                            pk_sb[32 * g:32 * g + 27, blk, :],
                            start=True, stop=True,
                            tile_position=(32 * g, 32 * g))
                nc.scalar.activation(c1sb[:, :, 0:16, 0:32], ps1h[0][:],
                                     AF.Relu, bias=bc1)
                nc.vector.tensor_scalar(c1sb[:, :, 16:32, 0:32], ps1h[1][:],
                                        bc1, 0.0, op0=OP.add, op1=OP.max)

                _pe_fill(KFILL)

                # ---- conv2: [32]->[48].  Images sharing a partition group
                # AND a tile position ({0,4} at (0,0), {1,5} at (32,64)) are
                # merged into single 512-col matmuls via the d-dim of c1sb:
                # 4 matmuls per tap instead of 6, and only 2 relu ops. ----
                if _do("c2"):
                  ps2a = pc_pool.tile([112, 2, 16, 16], F32, tag="psc",
                                      name="ps2a")
                  ps2b = pcb_pool.tile([112, 16, 16], F32, tag="pscb",
                                       name="ps2b")
                  for k, (dy, dx) in enumerate(
                          (dy, dx) for dy in range(3) for dx in range(3)):
                      st, sp = (k == 0), (k == 8)
                      wk = cb[:, CB_W2 + k * 48:CB_W2 + k * 48 + 48]
                      # images {0,4}: group 0, position (0,0)
                      nc.tensor.matmul(
                          ps2a[0:48, :, :, :], wk[0:32],
                          c1sb[0:32, :, dy:dy + 31:2, dx:dx + 31:2],
                          start=st, stop=sp, tile_position=(0, 0))
                      # images {1,5}: group 1, position (32,64)
                      nc.tensor.matmul(
                          ps2a[64:112, :, :, :], wk[32:64],
                          c1sb[32:64, :, dy:dy + 31:2, dx:dx + 31:2],
                          start=st, stop=sp, tile_position=(32, 64))
                      # image 2: group 2, position (64,0)
                      nc.tensor.matmul(
                          ps2b[0:48, :, :], wk[64:96],
                          c1sb[64:96, 0, dy:dy + 31:2, dx:dx + 31:2],
                          start=st, stop=sp, tile_position=(64, 0))
                      # image 3: group 3, position (96,64)
                      nc.tensor.matmul(
                          ps2b[64:112, :, :], wk[96:128],
                          c1sb[96:128, 0, dy:dy + 31:2, dx:dx + 31:2],
                          start=st, stop=sp, tile_position=(96, 64))
                  # ps2a rows 48:64 were never written; the matching c2sb
                  # rows are unused, so the relu may stream them as garbage.
                  nc.scalar.activation(c2sb[:, 0:3:2, 0:16, 0:16], ps2a[:],
                                       AF.Relu, bias=bc2)
                  nc.vector.tensor_scalar(c2sb[:, 1, 0:16, 0:16], ps2b[:],
                                          bc2, 0.0, op0=OP.add, op1=OP.max)

                _pe_fill(KFILL)

                # ---- conv3: [48]->[64], even/odd images in parallel
                # tiles (same column position -> separate PSUM banks).
                # featc image slots come out in order [0, 2, 4, 1, 3, 5]. ----
                if _do("c3"):
                  ps3a = ps_pool.tile([64, 3, D, D], F32, tag="sm")
                  ps3b = ps_pool.tile([64, 3, D, D], F32, tag="sm")
                  for k, (dy, dx) in enumerate(
                          (dy, dx) for dy in range(3) for dx in range(3)):
                      st, sp = (k == 0), (k == 8)
                      nc.tensor.matmul(
                          ps3a[:],
                          cb[0:48, CB_W3 + k * 64:CB_W3 + k * 64 + 64],
                          c2sb[0:48, :, dy:dy + 15:2, dx:dx + 15:2],
                          start=st, stop=sp, tile_position=(0, 0))
                      nc.tensor.matmul(
                          ps3b[:],
                          cb[64:112, CB_W3 + k * 64:CB_W3 + k * 64 + 64],
                          c2sb[64:112, :, dy:dy + 15:2, dx:dx + 15:2],
                          start=st, stop=sp, tile_position=(64, 0))
                  nc.scalar.activation(
                      featc[0:64, 0:192].rearrange("p (i m) -> p i m", m=M),
                      ps3a[:].rearrange("p i a b -> p i (a b)"),
                      AF.Relu, bias=bc3)
                  nc.vector.tensor_scalar(
                      featc[0:64, 192:384].rearrange("p (i m) -> p i m", m=M),
                      ps3b[:].rearrange("p i a b -> p i (a b)"),
                      bc3, 0.0, op0=OP.add, op1=OP.max)

                # ---- cls: per-image channel sums, rest on host.
                # The DVE reduce is emitted AFTER the hdd stream (DVE is the
                # relation bottleneck; in the tail it has idle cycles). ----
                fme = wpool.tile([64, S], F32)
                nc.gpsimd.memset(fme[:], 0.0)

                # ---- u / v ----
                u_f32 = wpool.tile([H1, S * M], F32)
                v_bf = wpool.tile([H1, S * M], BF16)
                if _do("uv"):
                    psu = ps_pool.tile([H1, S * M], F32, tag="sm")
                    psv = ps_pool.tile([H1, S * M], F32, tag="sm")
                    # v first: it gates every hdd op; u only gates per-column
                    nc.tensor.matmul(psv[:], w1b, featc[:], start=True, stop=True)
                    nc.tensor.matmul(psu[:], w1a, featc[:], start=True, stop=True)
                    # v on ACT, u on DVE: the two copies run concurrently
                    nc.scalar.activation(v_bf[:], psv[:], AF.Identity, bias=bg1)
                    nc.vector.tensor_copy(u_f32[:], psu[:])
                else:
                    nc.gpsimd.memset(u_f32[:], 0.0)
                    nc.gpsimd.memset(v_bf[:], 0.0)

            # ---- relation stage ----
            KH_ACT0 = int(os.environ.get("KH_ACT0", "12"))
            with tc.tile_pool(name="pbig", bufs=2, space="PSUM") as pb_pool:
              if _do("rel"):
                  # pre-allocate and manually rotate tiles: pool-per-iteration
                  # tiles each cost a TileRelease w/ cross-engine sync at exit
                  NHB = int(os.environ.get("KHB", "3"))
                  hdd_t = [hpool.tile([H1, 32, S * M], BF16, tag="hdd",
                                      name=f"hddt{i}") for i in range(NHB)]
                  ps_t = [pb_pool.tile([128, 2048], F32, tag="gps",
                                       name=f"pst{i}") for i in range(2)]
                  gscr_t = [spool.tile([128, 2048], BF16, tag="gscr",
                                       name=f"gscrt{i}") for i in range(3)]
                  psc = [0, 0]
                  # featc/u/v hold images in slot order [0, 2, 4, 1, 3, 5];
                  # local j-images 0,1,2 live in slots 0, 3, 1.
                  for jl, jsl in enumerate((0, 3, 1)):
                      for qh in range(2):
                          unit = jl * 2 + qh
                          hdd = hdd_t[unit % NHB]
                          # unit 5 keeps its hdd fully on DVE: ACT is gscr-
                          # backlogged at that point and would gate the final
                          # PE matmuls later than DVE does
                          n_act = (KH_ACT0 if unit == 0 else
                                   int(os.environ.get("KH_LAST", "0"))
                                   if unit == 5 else KH_ACT)
                          hdd_act = set(range(32 - n_act, 32))
                          for ql in range(32):
                              q = qh * 32 + ql
                              ucol = u_f32[:, jsl * M + q: jsl * M + q + 1]
                              if ql in hdd_act:
                                  nc.scalar.activation(hdd[:, ql, :], v_bf[:],
                                                       AF.Relu, bias=ucol)
                              else:
                                  nc.vector.tensor_scalar(hdd[:, ql, :], v_bf[:],
                                                          ucol, 0.0,
                                                          op0=OP.add, op1=OP.max)
                          if unit == 5 and _do("uv"):
                              # DVE idles briefly right after its final hdd op
                              # (PE still filling psum): slot the cls reduce in
                              nc.vector.tensor_reduce(
                                  fme[:],
                                  featc[0:64, :].rearrange(
                                      "p (i m) -> p i m", m=M),
                                  axis=mybir.AxisListType.X, op=OP.add)
                              nc.sync.dma_start(out=out_fme[:], in_=fme[:])
                          if unit == 0 and os.environ.get("KU0", "0") == "1":
                              for duo in range(3):
                                  iA, iB = 2 * duo, 2 * duo + 1
                                  col = unit * 3 + duo
                                  for hf in range(2):
                                      psh = ps_t[psc[0] % 2][:, 0:1024]
                                      psc[0] += 1
                                      for qg2 in range(2):
                                          qg = hf * 2 + qg2
                                          nc.tensor.matmul(
                                              psh[0:CO, qg2 * 512:(qg2 + 1) * 512],
                                              wg2,
                                              hdd[:, qg * 8:(qg + 1) * 8,
                                                  iA * M:(iA + 1) * M],
                                              start=True, stop=True,
                                              tile_position=(0, 0))
                                          nc.tensor.matmul(
                                              psh[CO:2 * CO, qg2 * 512:(qg2 + 1) * 512],
                                              wg2,
                                              hdd[:, qg * 8:(qg + 1) * 8,
                                                  iB * M:(iB + 1) * M],
                                              start=True, stop=True,
                                              tile_position=(0, 64))
                                      gscr = gscr_t[psc[1] % 3][:, 0:1024]
                                      psc[1] += 1
                                      xft = xf_a if hf == 0 else xf_d
                                      nc.scalar.activation(
                                          gscr[:], psh[:], AF.Relu, bias=bg2,
                                          accum_out=xft[:, col:col + 1])
                              continue
                          for duo in range(3):
                              iA, iB = 2 * duo, 2 * duo + 1
                              ps = ps_t[psc[0] % 2][:]
                              psc[0] += 1
                              for qg in range(4):
                                  nc.tensor.matmul(
                                      ps[0:CO, qg * 512:(qg + 1) * 512],
                                      wg2,
                                      hdd[:, qg * 8:(qg + 1) * 8, iA * M:(iA + 1) * M],
                                      start=True, stop=True,
                                      tile_position=(0, 0))
                                  nc.tensor.matmul(
                                      ps[CO:2 * CO, qg * 512:(qg + 1) * 512],
                                      wg2,
                                      hdd[:, qg * 8:(qg + 1) * 8, iB * M:(iB + 1) * M],
                                      start=True, stop=True,
                                      tile_position=(0, 64))
                              col = unit * 3 + duo
                              gscr = gscr_t[psc[1] % 3][:]
                              psc[1] += 1
                              if KSPLIT and unit == 5:
                                  # final tile: drain both halves concurrently
                                  nc.scalar.activation(
                                      gscr[:, 0:1024], ps[:, 0:1024],
                                      AF.Relu, bias=bg2,
                                      accum_out=xf_a[:, col:col + 1])
                                  nc.vector.scalar_tensor_tensor(
                                      gscr[:, 1024:2048], ps[:, 1024:2048],
                                      bg2, zb2048[:, 0:1024],
                                      op0=OP.add, op1=OP.max,
                                      accum_out=xf_d[:, col:col + 1])
                              elif duo < KG_V or (unit, duo) in KDVE:
                                  nc.vector.scalar_tensor_tensor(
                                      gscr[:], ps[:], bg2, zb2048[:],
                                      op0=OP.add, op1=OP.max,
                                      accum_out=xf_d[:, col:col + 1])
                              else:
                                  nc.scalar.activation(
                                      gscr[:], ps[:], AF.Relu, bias=bg2,
                                      accum_out=xf_a[:, col:col + 1])

            if not _do("uv"):
                nc.sync.dma_start(out=out_fme[:], in_=fme[:])
            nc.sync.dma_start(out=out_xf[:], in_=xf_all[:])
    nc.compile()
    return nc


_NC_CACHE = None


def _get_nc():
    global _NC_CACHE
    if _NC_CACHE is None:
        _NC_CACHE = _build_nc()
    return _NC_CACHE


def _host_prep(inputs):
    ins = {k: np.asarray(v) for k, v in inputs.items()}
    x = np.concatenate([ins['support_x'], ins['query_x']], axis=1)
    lab = np.concatenate([ins['support_y'], ins['query_y']], axis=1)

    xpad = np.pad(x.astype(np.float32), ((0, 0), (0, 0), (0, 0), (0, 1), (0, 1)))
    win = np.lib.stride_tricks.sliding_window_view(xpad, (3, 3), axis=(3, 4))
    win = win[:, :, :, ::2, ::2]
    patches = win.transpose(0, 2, 5, 6, 1, 3, 4).reshape(B, 27, S, 1024)
    patches = np.ascontiguousarray(patches, np.float32)

    f32 = np.float32
    bf16 = ml_dtypes.bfloat16

    w1 = ins['k1'].reshape(32, 27).T
    w2 = ins['k2'].transpose(1, 2, 3, 0).reshape(32, 432)
    w3 = ins['k3'].transpose(1, 2, 3, 0).reshape(48, 576)
    cb = np.zeros((128, CB_N), f32)
    for g in range(4):
        cb[32 * g:32 * g + 27, CB_W1:CB_W1 + 32] = w1
        cb[32 * g:32 * g + 32, CB_W2:CB_W2 + 432] = w2
    cb[0:48, CB_W3:CB_W3 + 576] = w3
    cb[64:112, CB_W3:CB_W3 + 576] = w3
    Wg1 = ins['Wg1'].astype(f32)
    cb[0:C2, CB_W1A:CB_W1A + H1] = Wg1[:C2]
    cb[0:C2, CB_W1B:CB_W1B + H1] = Wg1[C2:]
    cb[:, CB_WG2:CB_WG2 + CO] = ins['Wg2']
    ii = np.arange(D, dtype=f32) / D
    coord = np.stack([np.broadcast_to(ii[:, None], (D, D)),
                      np.broadcast_to(ii[None, :], (D, D))]).reshape(2, M)
    cb[0:2, CB_COORD:CB_COORD + 384] = np.tile(coord, (1, S))
    cb = cb.astype(bf16)

    cf = np.zeros((128, CF_N), f32)
    for g in range(4):
        cf[32 * g:32 * g + 32, 0] = ins['bc1']
    cf[0:48, 1] = ins['bc2']
    cf[64:112, 1] = ins['bc2']
    cf[0:64, 2] = ins['bc3']
    cf[:, 3] = ins['bg1']
    cf[:, 4] = np.tile(ins['bg2'].astype(f32), 2)

    in_maps = []
    for core in range(N_CORES):
        b, half = core // 2, core % 2
        perm = (0, 1, 2, 3, 4, 5) if half == 0 else (3, 4, 5, 0, 1, 2)
        pc = patches[b][:, perm, :]              # [27, S, 1024]
        pk = np.zeros((128, 4, 512), f32)
        for blk in range(4):
            h, d = blk % 2, blk // 2
            for g in range(4 if blk < 2 else 2):
                img = g + 4 * d
                pk[32 * g:32 * g + 27, blk, :] = pc[:, img, h * 512:(h + 1) * 512]
        m = dict(cb=cb, cf=cf, pk=pk.astype(bf16))
        in_maps.append(m)
    return in_maps, lab, ins


def _host_post(results, lab, ins):
    f64 = np.float64
    Wf1 = ins['Wf1'].astype(f64)
    bf1 = ins['bf1'].astype(f64)
    Wf2 = ins['Wf2'].astype(f64)
    bf2 = ins['bf2'].astype(f64)
    Wlog = ins['Wlog'].astype(f64)
    blog = ins['blog'].astype(f64)

    P = np.zeros((B, S, S), f64)
    cls_terms = np.zeros((B, S), f64)
    for core in range(N_CORES):
        b, half = core // 2, core % 2
        perm = (0, 1, 2, 3, 4, 5) if half == 0 else (3, 4, 5, 0, 1, 2)
        slot_img = (0, 2, 4, 1, 3, 5)                   # image held by slot s
        xfp = results[core]["xf"].astype(f64)           # [128, 38] packed
        xf = xfp[:, 0:18] + xfp[:, 20:38]               # [128, 18]
        xf9 = xf.reshape(128, 3, 2, 3).sum(axis=2)      # (jl, duo)
        for jl in range(3):
            for duo in range(3):
                for par in range(2):
                    i = slot_img[2 * duo + par]
                    vec = xf9[par * 64:(par + 1) * 64, jl, duo]
                    h = np.maximum(vec @ Wf1 + bf1, 0.0)
                    z = h @ Wf2 + bf2
                    P[b, perm[i], perm[jl]] = 1.0 / (1.0 + np.exp(-z[0]))
        if half == 0:
            fme = results[core]["fme"].astype(f64)      # [64, S] channel sums
            logits = (fme.T / M) @ Wlog + blog          # rows = slots
            mx = logits.max(axis=1)
            lse = mx + np.log(np.exp(logits - mx[:, None]).sum(axis=1))
            for s in range(S):
                img = slot_img[s]
                cls_terms[b, img] = lse[s] - logits[s, lab[b][img]]

    cls_loss = cls_terms.mean()
    y = (lab[:, :, None] == lab[:, None, :]).astype(f64)
    Pt = P.transpose(0, 2, 1)
    sym, anti = 0.5 * (P + Pt), 0.5 * (P - Pt)
    sym_n = np.sqrt((sym ** 2).sum(axis=(1, 2)))
    anti_n = np.sqrt((anti ** 2).sum(axis=(1, 2)))
    sym_loss = ((sym_n - anti_n) / (sym_n + anti_n)).mean()
    euc_loss = ((P - y) ** 2).mean()
    rn_loss = euc_loss - 0.1 * sym_loss
    return np.float32(cls_loss), np.float32(rn_loss), np.float32(sym_loss)


def run_spmd(inputs, trace=False, **kwargs):
    nc = _get_nc()
    in_maps, lab, ins = _host_prep(inputs)
    res = run_bass_kernel_spmd(nc, in_maps, list(range(N_CORES)),
                               trace=trace, **kwargs)
    return _host_post(res.results, lab, ins), res


def kernel(**inputs):
    out, _ = run_spmd(inputs)
    return out



# revision 45
# speedup vs baseline: 1.0020x; 1.0020x over previous
"""Trainium2 Bass kernel for nn_Meta_67078799229377 (relation-network meta-learner).

Sharding: 8 cores = 4 batch elements x 2 halves of the relation-j axis.
Each core runs the full backbone for its batch element's 6 images, then the
relation g-MLP for its 18 (i, j) pairs fully fused on-chip.  The device only
produces (a) per-image channel sums `fme` for the cls head and (b) the
(q,p)-summed relation features `xf`; the tiny f/cls MLP heads and loss
reductions run on the host in f64.

Engine plan (measured rates):
  - hdd = relu(v + u_q): DVE tensor_scalar, ~229ns i2i per [128,384]
    (2x perf mode; 1-op and 2-op variants measure identically, so the
    max(v,-u) factorization buys nothing).  A few units go to ACT
    (505ns each) to absorb its idle head.  This stream is the relation
    bottleneck: 178 ops x 229ns = 40.8us, LP-tight against ACT's gscr.
  - g matmuls: PE pairs at tile_position (0,0)/(0,64).  The PE runs at
    its mid p-state (~427ns/pair) throughout the relation phase: the
    full 2.4GHz state needs ~8us of unbroken execution which the
    psum-rotation stalls (drain 2.18us > fill 1.7us) never allow.
    That's fine steady-state (ACT-paced) and costs ~2.5us in the tail.
  - gscr relu+bias+sum: ACT activation w/ accum_out, 2.18us per
    [128,2048] tile incl the 283ns ACTIVATION_READ_ACCUMULATOR.
    The final two tiles drain split ACT || DVE halves (KSPLIT).
  - Input DMAs ride both HWDGE queues (sync + ACT) concurrently;
    conv1 start is bound by DMA completion latency (~10.5us).
  - gpsimd: memsets only (tensor ops crash this ucode build).
  - Teardown (~9.3us: 325 sem-reset/barrier instrs) is framework-fixed.
"""
import os
import numpy as np
import ml_dtypes

import concourse.bass as bass
import concourse.mybir as mybir
import concourse.tile as tile
from concourse import bacc
from concourse.bass_utils import run_bass_kernel_spmd

F32 = mybir.dt.float32
BF16 = mybir.dt.bfloat16
AF = mybir.ActivationFunctionType
OP = mybir.AluOpType

B, S, D = 4, 6, 8
M = D * D            # 64 spatial positions
C2 = 66              # 64 channels + 2 coord channels
H1 = 128             # g-MLP hidden
CO = 64              # g-MLP out
NCls = 64
N_CORES = 8

# bf16 const blob column layout
CB_W1 = 0            # [27, 32]
CB_W2 = 32           # [32, 432]
CB_W3 = 464          # [48, 576]
CB_W1A = 1040        # [66, 128]
CB_W1B = 1168        # [66, 128]
CB_WG2 = 1296        # [128, 64]
CB_COORD = 1360      # [2, 384]
CB_N = 1744

# f32 const blob column layout: bc1, bc2, bc3, bg1, bg2(x2)
CF_N = 5

KWARM = int(os.environ.get("KWARM", "5"))
KH_ACT = int(os.environ.get("KH_ACT", "1"))   # hdd instrs per unit on ACT
KG_V = int(os.environ.get("KG_V", "0"))       # gscr instrs per unit on DVE
KFILL = int(os.environ.get("KFILL", "2"))     # PE filler pairs per conv gap
# tail gscr tiles handled by DVE as "unit:duo" pairs (DVE is idle after the
# last hdd, ACT otherwise serializes the last unit's three tiles)
KDVE = {tuple(int(x) for x in ud.split(":"))
        for ud in os.environ.get("KDVE", "").split(",") if ud}
KSPLIT = os.environ.get("KSPLIT", "1") == "1"  # split the final tile's drain


def _build_nc():
    nc = bacc.Bacc("TRN2", target_bir_lowering=False, debug=False,
                   num_devices=N_CORES)

    x_pk = nc.dram_tensor("pk", [128, 4, 512], BF16, kind="ExternalInput")
    x_cb = nc.dram_tensor("cb", [128, CB_N], BF16, kind="ExternalInput")
    x_cf = nc.dram_tensor("cf", [128, CF_N], F32, kind="ExternalInput")

    out_fme = nc.dram_tensor("fme", [64, S], F32, kind="ExternalOutput")
    out_xf = nc.dram_tensor("xf", [128, 38], F32, kind="ExternalOutput")

    with tile.TileContext(nc) as tc:
        with (
            tc.tile_pool(name="const", bufs=1) as cpool,
            tc.tile_pool(name="work", bufs=1) as wpool,
            tc.tile_pool(name="patch", bufs=1) as ppool,
            tc.tile_pool(name="hdd", bufs=int(os.environ.get("KHB", "3"))) as hpool,
            tc.tile_pool(name="gscr", bufs=3) as spool,
        ):
            _stages = ["c1", "c2", "c3", "uv", "rel", "full"]
            _stop = os.environ.get("KSTOP", "full")
            def _do(stage):
                return _stages.index(stage) <= _stages.index(_stop)

            # ---- scratch + warmup (no input deps: runs during DMA) ----
            wsrc = cpool.tile([128, 512], BF16, tag="wsrc")
            nc.gpsimd.memset(wsrc[:], 0.0)
            ttrig = wpool.tile([128, 2], F32, tag="ttrig")

            cb = cpool.tile([128, CB_N], BF16, tag="cb")
            cf = cpool.tile([128, CF_N], F32, tag="cf")
            pk_sb = ppool.tile([128, 4, 512], BF16)
            # DMA order follows the consumption order: conv1 patches first
            # (conv1 is the head of the dependency chain), then conv weights,
            # then relation weights/biases.  Slices match contiguity in DRAM
            # (2KB+ per-partition chunks) to keep DMA at full rate.
            # two HWDGE queues: patches on the ACT queue run concurrently
            # with weights on the sync queue
            nc.scalar.dma_start(out=pk_sb[:, 0:2, :], in_=x_pk[:, 0:2, :])
            nc.sync.dma_start(out=cb[:, 0:CB_W1A], in_=x_cb[:, 0:CB_W1A])
            nc.scalar.dma_start(out=pk_sb[:, 2:4, :], in_=x_pk[:, 2:4, :])
            nc.sync.dma_start(out=cb[:, CB_W1A:], in_=x_cb[:, CB_W1A:])
            nc.sync.dma_start(out=cf[:], in_=x_cf[:])

            # trigger the ACT function-table load early (relu set)
            nc.scalar.activation(ttrig[:], wsrc[:, 0:2], AF.Relu)

            w1a = cb[0:C2, CB_W1A:CB_W1A + H1]
            w1b = cb[0:C2, CB_W1B:CB_W1B + H1]
            wg2 = cb[:, CB_WG2:CB_WG2 + CO]
            bc1 = cf[:, 0:1]        # replicated x4 partition groups
            bc2 = cf[0:112, 1:2]    # replicated at parts 0:48 and 64:112
            bc3 = cf[0:64, 2:3]
            bg1 = cf[:, 3:4]
            bg2 = cf[:, 4:5]

            featc = wpool.tile([C2, S * M], BF16)
            nc.vector.tensor_copy(featc[64:66, :], cb[0:2, CB_COORD:CB_COORD + 384])

            xf_all = wpool.tile([128, 38], F32, tag="xfall")
            nc.gpsimd.memset(xf_all[:], 0.0)
            xf_a = xf_all[:, 0:18]
            xf_d = xf_all[:, 20:38]

            zb2048 = cpool.tile([128, 2048], BF16, tag="zb")
            nc.gpsimd.memset(zb2048[:], 0.0)

            # c1sb[32*(i%4)+c, i//4, y, x] holds conv1 output of image i
            # c2sb[64*(i%2)+c, i//2, y, x] holds conv2 output of image i
            c1sb = wpool.tile([128, 2, 33, 33], BF16)
            c2sb = wpool.tile([112, 3, 17, 17], BF16)
            for d in range(2):
                nc.gpsimd.memset(c1sb[:, d, 32, :], 0.0)
                nc.gpsimd.memset(c1sb[:, d, 0:32, 32], 0.0)
            for d in range(3):
                nc.gpsimd.memset(c2sb[:, d, 16, :], 0.0)
                nc.gpsimd.memset(c2sb[:, d, 0:16, 16], 0.0)

            with (
                tc.tile_pool(name="pconv", bufs=2, space="PSUM") as pc_pool,
                tc.tile_pool(name="psmall", bufs=2, space="PSUM") as ps_pool,
                tc.tile_pool(name="pcb", bufs=1, space="PSUM") as pcb_pool,
                tc.tile_pool(name="pwarm", bufs=1, space="PSUM") as pw_pool,
            ):
                psw = pw_pool.tile([128, 512], F32, tag="warm")

                def _pe_fill(n):
                    # dependency-free matmuls that keep the PE clock ramped
                    # through relu-wait gaps between conv stages
                    for r in range(n):
                        nc.tensor.matmul(psw[0:64, :], wsrc[:, 0:64], wsrc[:],
                                         start=True, stop=True,
                                         tile_position=(0, 0))
                        nc.tensor.matmul(psw[64:128, :], wsrc[:, 0:64],
                                         wsrc[:],
                                         start=True, stop=True,
                                         tile_position=(0, 64))

                _pe_fill(KWARM)

                # ---- conv1: [27]->[32], 4 images per matmul round via
                # tile_position packing (diagonal 32x32 tiles).  Both d-halves
                # of each y-half share one psum tile so the epilogue is two
                # big concurrent relus (ACT || DVE) instead of four.
                # pA/pB rows 64:128 at d=1 are never written; the matching
                # c1sb region is unused, so streaming garbage there is fine.
                ps1h = [pc_pool.tile([128, 2, 16, 32], F32, tag="psc",
                                     name=f"ps1h{h}") for h in range(2)]
                for blk in range(4):
                    groups = (0, 1, 2, 3) if blk < 2 else (0, 1)
                    h, d = blk % 2, blk // 2
                    for g in groups:
                        nc.tensor.matmul(
                            ps1h[h][32 * g:32 * g + 32, d, :, :].rearrange(
                                "p a b -> p (a b)"),
                            cb[32 * g:32 * g + 27, CB_W1:CB_W1 + 32],
                            pk_sb[32 * g:32 * g + 27, blk, :],
                            start=True, stop=True,
                            tile_position=(32 * g, 32 * g))
                nc.scalar.activation(c1sb[:, :, 0:16, 0:32], ps1h[0][:],
                                     AF.Relu, bias=bc1)
                nc.vector.tensor_scalar(c1sb[:, :, 16:32, 0:32], ps1h[1][:],
                                        bc1, 0.0, op0=OP.add, op1=OP.max)

                _pe_fill(KFILL)

                # ---- conv2: [32]->[48].  Images sharing a partition group
                # AND a tile position ({0,4} at (0,0), {1,5} at (32,64)) are
                # merged into single 512-col matmuls via the d-dim of c1sb:
                # 4 matmuls per tap instead of 6, and only 2 relu ops. ----
                if _do("c2"):
                  ps2a = pc_pool.tile([112, 2, 16, 16], F32, tag="psc",
                                      name="ps2a")
                  ps2b = pcb_pool.tile([112, 16, 16], F32, tag="pscb",
                                       name="ps2b")
                  for k, (dy, dx) in enumerate(
                          (dy, dx) for dy in range(3) for dx in range(3)):
                      st, sp = (k == 0), (k == 8)
                      wk = cb[:, CB_W2 + k * 48:CB_W2 + k * 48 + 48]
                      # images {0,4}: group 0, position (0,0)
                      nc.tensor.matmul(
                          ps2a[0:48, :, :, :], wk[0:32],
                          c1sb[0:32, :, dy:dy + 31:2, dx:dx + 31:2],
                          start=st, stop=sp, tile_position=(0, 0))
                      # images {1,5}: group 1, position (32,64)
                      nc.tensor.matmul(
                          ps2a[64:112, :, :, :], wk[32:64],
                          c1sb[32:64, :, dy:dy + 31:2, dx:dx + 31:2],
                          start=st, stop=sp, tile_position=(32, 64))
                      # image 2: group 2, position (64,0)
                      nc.tensor.matmul(
                          ps2b[0:48, :, :], wk[64:96],
                          c1sb[64:96, 0, dy:dy + 31:2, dx:dx + 31:2],
                          start=st, stop=sp, tile_position=(64, 0))
                      # image 3: group 3, position (96,64)
                      nc.tensor.matmul(
                          ps2b[64:112, :, :], wk[96:128],
                          c1sb[96:128, 0, dy:dy + 31:2, dx:dx + 31:2],
                          start=st, stop=sp, tile_position=(96, 64))
                  # ps2a rows 48:64 were never written; the matching c2sb
                  # rows are unused, so the relu may stream them as garbage.
                  nc.scalar.activation(c2sb[:, 0:3:2, 0:16, 0:16], ps2a[:],
                                       AF.Relu, bias=bc2)
                  nc.vector.tensor_scalar(c2sb[:, 1, 0:16, 0:16], ps2b[:],
                                          bc2, 0.0, op0=OP.add, op1=OP.max)

                _pe_fill(KFILL)

                # ---- conv3: [48]->[64], even/odd images in parallel
                # tiles (same column position -> separate PSUM banks).
                # featc image slots come out in order [0, 2, 4, 1, 3, 5]. ----
                if _do("c3"):
                  ps3a = ps_pool.tile([64, 3, D, D], F32, tag="sm")
                  ps3b = ps_pool.tile([64, 3, D, D], F32, tag="sm")
                  for k, (dy, dx) in enumerate(
                          (dy, dx) for dy in range(3) for dx in range(3)):
                      st, sp = (k == 0), (k == 8)
                      nc.tensor.matmul(
                          ps3a[:],
                          cb[0:48, CB_W3 + k * 64:CB_W3 + k * 64 + 64],
                          c2sb[0:48, :, dy:dy + 15:2, dx:dx + 15:2],
                          start=st, stop=sp, tile_position=(0, 0))
                      nc.tensor.matmul(
                          ps3b[:],
                          cb[64:112, CB_W3 + k * 64:CB_W3 + k * 64 + 64],
                          c2sb[64:112, :, dy:dy + 15:2, dx:dx + 15:2],
                          start=st, stop=sp, tile_position=(64, 0))
                  nc.scalar.activation(
                      featc[0:64, 0:192].rearrange("p (i m) -> p i m", m=M),
                      ps3a[:].rearrange("p i a b -> p i (a b)"),
                      AF.Relu, bias=bc3)
                  nc.vector.tensor_scalar(
                      featc[0:64, 192:384].rearrange("p (i m) -> p i m", m=M),
                      ps3b[:].rearrange("p i a b -> p i (a b)"),
                      bc3, 0.0, op0=OP.add, op1=OP.max)

                # ---- cls: per-image channel sums, rest on host.
                # The DVE reduce is emitted AFTER the hdd stream (DVE is the
                # relation bottleneck; in the tail it has idle cycles). ----
                fme = wpool.tile([64, S], F32)
                nc.gpsimd.memset(fme[:], 0.0)

                # ---- u / v ----
                u_f32 = wpool.tile([H1, S * M], F32)
                v_bf = wpool.tile([H1, S * M], BF16)
                if _do("uv"):
                    psu = ps_pool.tile([H1, S * M], F32, tag="sm")
                    psv = ps_pool.tile([H1, S * M], F32, tag="sm")
                    # v first: it gates every hdd op; u only gates per-column
                    nc.tensor.matmul(psv[:], w1b, featc[:], start=True, stop=True)
                    nc.tensor.matmul(psu[:], w1a, featc[:], start=True, stop=True)
                    # v on ACT, u on DVE: the two copies run concurrently
                    nc.scalar.activation(v_bf[:], psv[:], AF.Identity, bias=bg1)
                    nc.vector.tensor_copy(u_f32[:], psu[:])
                else:
                    nc.gpsimd.memset(u_f32[:], 0.0)
                    nc.gpsimd.memset(v_bf[:], 0.0)

            # ---- relation stage ----
            KH_ACT0 = int(os.environ.get("KH_ACT0", "12"))
            with tc.tile_pool(name="pbig", bufs=2, space="PSUM") as pb_pool:
              if _do("rel"):
                  # pre-allocate and manually rotate tiles: pool-per-iteration
                  # tiles each cost a TileRelease w/ cross-engine sync at exit
                  NHB = int(os.environ.get("KHB", "3"))
                  hdd_t = [hpool.tile([H1, 32, S * M], BF16, tag="hdd",
                                      name=f"hddt{i}") for i in range(NHB)]
                  ps_t = [pb_pool.tile([128, 2048], F32, tag="gps",
                                       name=f"pst{i}") for i in range(2)]
                  gscr_t = [spool.tile([128, 2048], BF16, tag="gscr",
                                       name=f"gscrt{i}") for i in range(3)]
                  psc = [0, 0]
                  # featc/u/v hold images in slot order [0, 2, 4, 1, 3, 5];
                  # local j-images 0,1,2 live in slots 0, 3, 1.
                  for jl, jsl in enumerate((0, 3, 1)):
                      for qh in range(2):
                          unit = jl * 2 + qh
                          hdd = hdd_t[unit % NHB]
                          # unit 5 keeps its hdd fully on DVE: ACT is gscr-
                          # backlogged at that point and would gate the final
                          # PE matmuls later than DVE does
                          n_act = (KH_ACT0 if unit == 0 else
                                   int(os.environ.get("KH_LAST", "1"))
                                   if unit == 5 else KH_ACT)
                          hdd_act = set(range(32 - n_act, 32))
                          for ql in range(32):
                              q = qh * 32 + ql
                              ucol = u_f32[:, jsl * M + q: jsl * M + q + 1]
                              if ql in hdd_act:
                                  nc.scalar.activation(hdd[:, ql, :], v_bf[:],
                                                       AF.Relu, bias=ucol)
                              else:
                                  nc.vector.tensor_scalar(hdd[:, ql, :], v_bf[:],
                                                          ucol, 0.0,
                                                          op0=OP.add, op1=OP.max)
                          if unit == 5 and _do("uv"):
                              # DVE idles briefly right after its final hdd op
                              # (PE still filling psum): slot the cls reduce in
                              nc.vector.tensor_reduce(
                                  fme[:],
                                  featc[0:64, :].rearrange(
                                      "p (i m) -> p i m", m=M),
                                  axis=mybir.AxisListType.X, op=OP.add)
                              nc.sync.dma_start(out=out_fme[:], in_=fme[:])
                          if unit == 0 and os.environ.get("KU0", "0") == "1":
                              for duo in range(3):
                                  iA, iB = 2 * duo, 2 * duo + 1
                                  col = unit * 3 + duo
                                  for hf in range(2):
                                      psh = ps_t[psc[0] % 2][:, 0:1024]
                                      psc[0] += 1
                                      for qg2 in range(2):
                                          qg = hf * 2 + qg2
                                          nc.tensor.matmul(
                                              psh[0:CO, qg2 * 512:(qg2 + 1) * 512],
                                              wg2,
                                              hdd[:, qg * 8:(qg + 1) * 8,
                                                  iA * M:(iA + 1) * M],
                                              start=True, stop=True,
                                              tile_position=(0, 0))
                                          nc.tensor.matmul(
                                              psh[CO:2 * CO, qg2 * 512:(qg2 + 1) * 512],
                                              wg2,
                                              hdd[:, qg * 8:(qg + 1) * 8,
                                                  iB * M:(iB + 1) * M],
                                              start=True, stop=True,
                                              tile_position=(0, 64))
                                      gscr = gscr_t[psc[1] % 3][:, 0:1024]
                                      psc[1] += 1
                                      xft = xf_a if hf == 0 else xf_d
                                      nc.scalar.activation(
                                          gscr[:], psh[:], AF.Relu, bias=bg2,
                                          accum_out=xft[:, col:col + 1])
                              continue
                          for duo in range(3):
                              iA, iB = 2 * duo, 2 * duo + 1
                              ps = ps_t[psc[0] % 2][:]
                              psc[0] += 1
                              for qg in range(4):
                                  nc.tensor.matmul(
                                      ps[0:CO, qg * 512:(qg + 1) * 512],
                                      wg2,
                                      hdd[:, qg * 8:(qg + 1) * 8, iA * M:(iA + 1) * M],
                                      start=True, stop=True,
                                      tile_position=(0, 0))
                                  nc.tensor.matmul(
                                      ps[CO:2 * CO, qg * 512:(qg + 1) * 512],
                                      wg2,
                                      hdd[:, qg * 8:(qg + 1) * 8, iB * M:(iB + 1) * M],
                                      start=True, stop=True,
                                      tile_position=(0, 64))
                              col = unit * 3 + duo
                              gscr = gscr_t[psc[1] % 3][:]
                              psc[1] += 1
                              if KSPLIT and unit == 5:
                                  # final tile: drain both halves concurrently
                                  nc.scalar.activation(
                                      gscr[:, 0:1024], ps[:, 0:1024],
                                      AF.Relu, bias=bg2,
                                      accum_out=xf_a[:, col:col + 1])
                                  nc.vector.scalar_tensor_tensor(
                                      gscr[:, 1024:2048], ps[:, 1024:2048],
                                      bg2, zb2048[:, 0:1024],
                                      op0=OP.add, op1=OP.max,
                                      accum_out=xf_d[:, col:col + 1])
                              elif duo < KG_V or (unit, duo) in KDVE:
                                  nc.vector.scalar_tensor_tensor(
                                      gscr[:], ps[:], bg2, zb2048[:],
                                      op0=OP.add, op1=OP.max,
                                      accum_out=xf_d[:, col:col + 1])
                              else:
                                  nc.scalar.activation(
                                      gscr[:], ps[:], AF.Relu, bias=bg2,
                                      accum_out=xf_a[:, col:col + 1])

            if not _do("uv"):
                nc.sync.dma_start(out=out_fme[:], in_=fme[:])
            nc.sync.dma_start(out=out_xf[:], in_=xf_all[:])
    nc.compile()
    return nc


_NC_CACHE = None


def _get_nc():
    global _NC_CACHE
    if _NC_CACHE is None:
        _NC_CACHE = _build_nc()
    return _NC_CACHE


def _host_prep(inputs):
    ins = {k: np.asarray(v) for k, v in inputs.items()}
    x = np.concatenate([ins['support_x'], ins['query_x']], axis=1)
    lab = np.concatenate([ins['support_y'], ins['query_y']], axis=1)

    xpad = np.pad(x.astype(np.float32), ((0, 0), (0, 0), (0, 0), (0, 1), (0, 1)))
    win = np.lib.stride_tricks.sliding_window_view(xpad, (3, 3), axis=(3, 4))
    win = win[:, :, :, ::2, ::2]
    patches = win.transpose(0, 2, 5, 6, 1, 3, 4).reshape(B, 27, S, 1024)
    patches = np.ascontiguousarray(patches, np.float32)

    f32 = np.float32
    bf16 = ml_dtypes.bfloat16

    w1 = ins['k1'].reshape(32, 27).T
    w2 = ins['k2'].transpose(1, 2, 3, 0).reshape(32, 432)
    w3 = ins['k3'].transpose(1, 2, 3, 0).reshape(48, 576)
    cb = np.zeros((128, CB_N), f32)
    for g in range(4):
        cb[32 * g:32 * g + 27, CB_W1:CB_W1 + 32] = w1
        cb[32 * g:32 * g + 32, CB_W2:CB_W2 + 432] = w2
    cb[0:48, CB_W3:CB_W3 + 576] = w3
    cb[64:112, CB_W3:CB_W3 + 576] = w3
    Wg1 = ins['Wg1'].astype(f32)
    cb[0:C2, CB_W1A:CB_W1A + H1] = Wg1[:C2]
    cb[0:C2, CB_W1B:CB_W1B + H1] = Wg1[C2:]
    cb[:, CB_WG2:CB_WG2 + CO] = ins['Wg2']
    ii = np.arange(D, dtype=f32) / D
    coord = np.stack([np.broadcast_to(ii[:, None], (D, D)),
                      np.broadcast_to(ii[None, :], (D, D))]).reshape(2, M)
    cb[0:2, CB_COORD:CB_COORD + 384] = np.tile(coord, (1, S))
    cb = cb.astype(bf16)

    cf = np.zeros((128, CF_N), f32)
    for g in range(4):
        cf[32 * g:32 * g + 32, 0] = ins['bc1']
    cf[0:48, 1] = ins['bc2']
    cf[64:112, 1] = ins['bc2']
    cf[0:64, 2] = ins['bc3']
    cf[:, 3] = ins['bg1']
    cf[:, 4] = np.tile(ins['bg2'].astype(f32), 2)

    in_maps = []
    for core in range(N_CORES):
        b, half = core // 2, core % 2
        perm = (0, 1, 2, 3, 4, 5) if half == 0 else (3, 4, 5, 0, 1, 2)
        pc = patches[b][:, perm, :]              # [27, S, 1024]
        pk = np.zeros((128, 4, 512), f32)
        for blk in range(4):
            h, d = blk % 2, blk // 2
            for g in range(4 if blk < 2 else 2):
                img = g + 4 * d
                pk[32 * g:32 * g + 27, blk, :] = pc[:, img, h * 512:(h + 1) * 512]
        m = dict(cb=cb, cf=cf, pk=pk.astype(bf16))
        in_maps.append(m)
    return in_maps, lab, ins


def _host_post(results, lab, ins):
    f64 = np.float64
    Wf1 = ins['Wf1'].astype(f64)
    bf1 = ins['bf1'].astype(f64)
    Wf2 = ins['Wf2'].astype(f64)
    bf2 = ins['bf2'].astype(f64)
    Wlog = ins['Wlog'].astype(f64)
    blog = ins['blog'].astype(f64)

    P = np.zeros((B, S, S), f64)
    cls_terms = np.zeros((B, S), f64)
    for core in range(N_CORES):
        b, half = core // 2, core % 2
        perm = (0, 1, 2, 3, 4, 5) if half == 0 else (3, 4, 5, 0, 1, 2)
        slot_img = (0, 2, 4, 1, 3, 5)                   # image held by slot s
        xfp = results[core]["xf"].astype(f64)           # [128, 38] packed
        xf = xfp[:, 0:18] + xfp[:, 20:38]               # [128, 18]
        xf9 = xf.reshape(128, 3, 2, 3).sum(axis=2)      # (jl, duo)
        for jl in range(3):
            for duo in range(3):
                for par in range(2):
                    i = slot_img[2 * duo + par]
                    vec = xf9[par * 64:(par + 1) * 64, jl, duo]
                    h = np.maximum(vec @ Wf1 + bf1, 0.0)
                    z = h @ Wf2 + bf2
                    P[b, perm[i], perm[jl]] = 1.0 / (1.0 + np.exp(-z[0]))
        if half == 0:
            fme = results[core]["fme"].astype(f64)      # [64, S] channel sums
            logits = (fme.T / M) @ Wlog + blog          # rows = slots
            mx = logits.max(axis=1)
            lse = mx + np.log(np.exp(logits - mx[:, None]).sum(axis=1))
            for s in range(S):
                img = slot_img[s]
                cls_terms[b, img] = lse[s] - logits[s, lab[b][img]]

    cls_loss = cls_terms.mean()
    y = (lab[:, :, None] == lab[:, None, :]).astype(f64)
    Pt = P.transpose(0, 2, 1)
    sym, anti = 0.5 * (P + Pt), 0.5 * (P - Pt)
    sym_n = np.sqrt((sym ** 2).sum(axis=(1, 2)))
    anti_n = np.sqrt((anti ** 2).sum(axis=(1, 2)))
    sym_loss = ((sym_n - anti_n) / (sym_n + anti_n)).mean()
    euc_loss = ((P - y) ** 2).mean()
    rn_loss = euc_loss - 0.1 * sym_loss
    return np.float32(cls_loss), np.float32(rn_loss), np.float32(sym_loss)


def run_spmd(inputs, trace=False, **kwargs):
    nc = _get_nc()
    in_maps, lab, ins = _host_prep(inputs)
    res = run_bass_kernel_spmd(nc, in_maps, list(range(N_CORES)),
                               trace=trace, **kwargs)
    return _host_post(res.results, lab, ins), res


def kernel(**inputs):
    out, _ = run_spmd(inputs)
    return out



# revision 46
# speedup vs baseline: 1.0327x; 1.0306x over previous
"""Trainium2 Bass kernel for nn_Meta_67078799229377 (relation-network meta-learner).

Sharding: 8 cores = 4 batch elements x 2 halves of the relation-j axis.
Each core runs the full backbone for its batch element's 6 images, then the
relation g-MLP for its 18 (i, j) pairs fully fused on-chip.  The device only
produces (a) per-image channel sums `fme` for the cls head and (b) the
(q,p)-summed relation features `xf`; the tiny f/cls MLP heads and loss
reductions run on the host in f64.

Engine plan (measured rates):
  - hdd = relu(v + u_q): DVE tensor_scalar, ~229ns i2i per [128,384]
    (2x perf mode; 1-op and 2-op variants measure identically, so the
    max(v,-u) factorization buys nothing).  A few units go to ACT
    (505ns each) to absorb its idle head.  This stream is the relation
    bottleneck: 178 ops x 229ns = 40.8us, LP-tight against ACT's gscr.
  - g matmuls: PE pairs at tile_position (0,0)/(0,64).  The PE runs at
    its mid p-state (~427ns/pair) throughout the relation phase: the
    full 2.4GHz state needs ~8us of unbroken execution which the
    psum-rotation stalls (drain 2.18us > fill 1.7us) never allow.
    That's fine steady-state (ACT-paced) and costs ~2.5us in the tail.
  - gscr relu+bias+sum: ACT activation w/ accum_out, 2.18us per
    [128,2048] tile incl the 283ns ACTIVATION_READ_ACCUMULATOR.
    The final two tiles drain split ACT || DVE halves (KSPLIT).
  - Input DMAs ride both HWDGE queues (sync + ACT) concurrently;
    conv1 start is bound by DMA completion latency (~10.5us).
  - gpsimd: memsets only (tensor ops crash this ucode build).
  - Teardown (~9.3us: 325 sem-reset/barrier instrs) is framework-fixed.
"""
import os
import numpy as np
import ml_dtypes

import concourse.bass as bass
import concourse.mybir as mybir
import concourse.tile as tile
from concourse import bacc
from concourse.bass_utils import run_bass_kernel_spmd

F32 = mybir.dt.float32
BF16 = mybir.dt.bfloat16
AF = mybir.ActivationFunctionType
OP = mybir.AluOpType

B, S, D = 4, 6, 8
M = D * D            # 64 spatial positions
C2 = 66              # 64 channels + 2 coord channels
H1 = 128             # g-MLP hidden
CO = 64              # g-MLP out
NCls = 64
N_CORES = 8

# bf16 const blob column layout
CB_W1 = 0            # [27, 32]
CB_W2 = 32           # [32, 432]
CB_W3 = 464          # [48, 576]
CB_W1A = 1040        # [66, 128]
CB_W1B = 1168        # [66, 128]
CB_WG2 = 1296        # [128, 64]
CB_COORD = 1360      # [2, 384]
CB_N = 1744

# f32 const blob column layout: bc1, bc2, bc3, bg1, bg2(x2)
CF_N = 5

KWARM = int(os.environ.get("KWARM", "5"))
KH_ACT = int(os.environ.get("KH_ACT", "1"))   # hdd instrs per unit on ACT
KG_V = int(os.environ.get("KG_V", "0"))       # gscr instrs per unit on DVE
KFILL = int(os.environ.get("KFILL", "2"))     # PE filler pairs per conv gap
# tail gscr tiles handled by DVE as "unit:duo" pairs (DVE is idle after the
# last hdd, ACT otherwise serializes the last unit's three tiles)
KDVE = {tuple(int(x) for x in ud.split(":"))
        for ud in os.environ.get("KDVE", "").split(",") if ud}
KSPLIT = os.environ.get("KSPLIT", "1") == "1"  # split the final tile's drain


def _build_nc():
    nc = bacc.Bacc("TRN2", target_bir_lowering=False, debug=False,
                   num_devices=N_CORES)

    x_pk = nc.dram_tensor("pk", [128, 4, 512], BF16, kind="ExternalInput")
    x_cb = nc.dram_tensor("cb", [128, CB_N], BF16, kind="ExternalInput")
    x_cf = nc.dram_tensor("cf", [128, CF_N], F32, kind="ExternalInput")

    out_fme = nc.dram_tensor("fme", [64, S], F32, kind="ExternalOutput")
    out_xf = nc.dram_tensor("xf", [128, 38], F32, kind="ExternalOutput")

    with tile.TileContext(nc) as tc:
        with (
            tc.tile_pool(name="const", bufs=1) as cpool,
            tc.tile_pool(name="work", bufs=1) as wpool,
            tc.tile_pool(name="patch", bufs=1) as ppool,
            tc.tile_pool(name="hdd", bufs=int(os.environ.get("KHB", "3"))) as hpool,
            tc.tile_pool(name="gscr", bufs=3) as spool,
        ):
            _stages = ["c1", "c2", "c3", "uv", "rel", "full"]
            _stop = os.environ.get("KSTOP", "full")
            def _do(stage):
                return _stages.index(stage) <= _stages.index(_stop)

            # ---- scratch + warmup (no input deps: runs during DMA) ----
            wsrc = cpool.tile([128, 512], BF16, tag="wsrc")
            nc.gpsimd.memset(wsrc[:], 0.0)
            ttrig = wpool.tile([128, 2], F32, tag="ttrig")

            cb = cpool.tile([128, CB_N], BF16, tag="cb")
            cf = cpool.tile([128, CF_N], F32, tag="cf")
            pk_sb = ppool.tile([128, 4, 512], BF16)
            # DMA order follows the consumption order: conv1 patches first
            # (conv1 is the head of the dependency chain), then conv weights,
            # then relation weights/biases.  Slices match contiguity in DRAM
            # (2KB+ per-partition chunks) to keep DMA at full rate.
            # two HWDGE queues: patches on the ACT queue run concurrently
            # with weights on the sync queue
            nc.scalar.dma_start(out=pk_sb[:, 0:2, :], in_=x_pk[:, 0:2, :])
            nc.sync.dma_start(out=cb[:, 0:CB_W1A], in_=x_cb[:, 0:CB_W1A])
            nc.scalar.dma_start(out=pk_sb[:, 2:4, :], in_=x_pk[:, 2:4, :])
            nc.sync.dma_start(out=cb[:, CB_W1A:], in_=x_cb[:, CB_W1A:])
            nc.sync.dma_start(out=cf[:], in_=x_cf[:])

            # trigger the ACT function-table load early (relu set)
            nc.scalar.activation(ttrig[:], wsrc[:, 0:2], AF.Relu)

            w1a = cb[0:C2, CB_W1A:CB_W1A + H1]
            w1b = cb[0:C2, CB_W1B:CB_W1B + H1]
            wg2 = cb[:, CB_WG2:CB_WG2 + CO]
            bc1 = cf[:, 0:1]        # replicated x4 partition groups
            bc2 = cf[0:112, 1:2]    # replicated at parts 0:48 and 64:112
            bc3 = cf[0:64, 2:3]
            bg1 = cf[:, 3:4]
            bg2 = cf[:, 4:5]

            featc = wpool.tile([C2, S * M], BF16)
            nc.vector.tensor_copy(featc[64:66, :], cb[0:2, CB_COORD:CB_COORD + 384])

            xf_all = wpool.tile([128, 38], F32, tag="xfall")
            nc.gpsimd.memset(xf_all[:], 0.0)
            xf_a = xf_all[:, 0:18]
            xf_d = xf_all[:, 20:38]

            zb2048 = cpool.tile([128, 2048], BF16, tag="zb")
            nc.gpsimd.memset(zb2048[:], 0.0)

            # c1sb[32*(i%4)+c, i//4, y, x] holds conv1 output of image i
            # c2sb[64*(i%2)+c, i//2, y, x] holds conv2 output of image i
            c1sb = wpool.tile([128, 2, 33, 33], BF16)
            c2sb = wpool.tile([112, 3, 17, 17], BF16)
            for d in range(2):
                nc.gpsimd.memset(c1sb[:, d, 32, :], 0.0)
                nc.gpsimd.memset(c1sb[:, d, 0:32, 32], 0.0)
            for d in range(3):
                nc.gpsimd.memset(c2sb[:, d, 16, :], 0.0)
                nc.gpsimd.memset(c2sb[:, d, 0:16, 16], 0.0)

            with (
                tc.tile_pool(name="pconv", bufs=2, space="PSUM") as pc_pool,
                tc.tile_pool(name="psmall", bufs=2, space="PSUM") as ps_pool,
                tc.tile_pool(name="pcb", bufs=1, space="PSUM") as pcb_pool,
                tc.tile_pool(name="pwarm", bufs=1, space="PSUM") as pw_pool,
            ):
                psw = pw_pool.tile([128, 512], F32, tag="warm")

                def _pe_fill(n):
                    # dependency-free matmuls that keep the PE clock ramped
                    # through relu-wait gaps between conv stages
                    for r in range(n):
                        nc.tensor.matmul(psw[0:64, :], wsrc[:, 0:64], wsrc[:],
                                         start=True, stop=True,
                                         tile_position=(0, 0))
                        nc.tensor.matmul(psw[64:128, :], wsrc[:, 0:64],
                                         wsrc[:],
                                         start=True, stop=True,
                                         tile_position=(0, 64))

                _pe_fill(KWARM)

                # ---- conv1: [27]->[32], 4 images per matmul round via
                # tile_position packing (diagonal 32x32 tiles).  Both d-halves
                # of each y-half share one psum tile so the epilogue is two
                # big concurrent relus (ACT || DVE) instead of four.
                # pA/pB rows 64:128 at d=1 are never written; the matching
                # c1sb region is unused, so streaming garbage there is fine.
                ps1h = [pc_pool.tile([128, 2, 16, 32], F32, tag="psc",
                                     name=f"ps1h{h}") for h in range(2)]
                for blk in range(4):
                    groups = (0, 1, 2, 3) if blk < 2 else (0, 1)
                    h, d = blk % 2, blk // 2
                    for g in groups:
                        nc.tensor.matmul(
                            ps1h[h][32 * g:32 * g + 32, d, :, :].rearrange(
                                "p a b -> p (a b)"),
                            cb[32 * g:32 * g + 27, CB_W1:CB_W1 + 32],
                            pk_sb[32 * g:32 * g + 27, blk, :],
                            start=True, stop=True,
                            tile_position=(32 * g, 32 * g))
                nc.scalar.activation(c1sb[:, :, 0:16, 0:32], ps1h[0][:],
                                     AF.Relu, bias=bc1)
                nc.vector.tensor_scalar(c1sb[:, :, 16:32, 0:32], ps1h[1][:],
                                        bc1, 0.0, op0=OP.add, op1=OP.max)

                _pe_fill(KFILL)

                # ---- conv2: [32]->[48].  Images sharing a partition group
                # AND a tile position ({0,4} at (0,0), {1,5} at (32,64)) are
                # merged into single 512-col matmuls via the d-dim of c1sb:
                # 4 matmuls per tap instead of 6, and only 2 relu ops. ----
                if _do("c2"):
                  ps2a = pc_pool.tile([112, 2, 16, 16], F32, tag="psc",
                                      name="ps2a")
                  ps2b = pcb_pool.tile([112, 16, 16], F32, tag="pscb",
                                       name="ps2b")
                  for k, (dy, dx) in enumerate(
                          (dy, dx) for dy in range(3) for dx in range(3)):
                      st, sp = (k == 0), (k == 8)
                      wk = cb[:, CB_W2 + k * 48:CB_W2 + k * 48 + 48]
                      # images {0,4}: group 0, position (0,0)
                      nc.tensor.matmul(
                          ps2a[0:48, :, :, :], wk[0:32],
                          c1sb[0:32, :, dy:dy + 31:2, dx:dx + 31:2],
                          start=st, stop=sp, tile_position=(0, 0))
                      # images {1,5}: group 1, position (32,64)
                      nc.tensor.matmul(
                          ps2a[64:112, :, :, :], wk[32:64],
                          c1sb[32:64, :, dy:dy + 31:2, dx:dx + 31:2],
                          start=st, stop=sp, tile_position=(32, 64))
                      # image 2: group 2, position (64,0)
                      nc.tensor.matmul(
                          ps2b[0:48, :, :], wk[64:96],
                          c1sb[64:96, 0, dy:dy + 31:2, dx:dx + 31:2],
                          start=st, stop=sp, tile_position=(64, 0))
                      # image 3: group 3, position (96,64)
                      nc.tensor.matmul(
                          ps2b[64:112, :, :], wk[96:128],
                          c1sb[96:128, 0, dy:dy + 31:2, dx:dx + 31:2],
                          start=st, stop=sp, tile_position=(96, 64))
                  # ps2a rows 48:64 were never written; the matching c2sb
                  # rows are unused, so the relu may stream them as garbage.
                  nc.scalar.activation(c2sb[:, 0:3:2, 0:16, 0:16], ps2a[:],
                                       AF.Relu, bias=bc2)
                  nc.vector.tensor_scalar(c2sb[:, 1, 0:16, 0:16], ps2b[:],
                                          bc2, 0.0, op0=OP.add, op1=OP.max)

                _pe_fill(KFILL)

                # ---- conv3: [48]->[64], even/odd images in parallel
                # tiles (same column position -> separate PSUM banks).
                # featc image slots come out in order [0, 2, 4, 1, 3, 5]. ----
                if _do("c3"):
                  ps3a = ps_pool.tile([64, 3, D, D], F32, tag="sm")
                  ps3b = ps_pool.tile([64, 3, D, D], F32, tag="sm")
                  for k, (dy, dx) in enumerate(
                          (dy, dx) for dy in range(3) for dx in range(3)):
                      st, sp = (k == 0), (k == 8)
                      nc.tensor.matmul(
                          ps3a[:],
                          cb[0:48, CB_W3 + k * 64:CB_W3 + k * 64 + 64],
                          c2sb[0:48, :, dy:dy + 15:2, dx:dx + 15:2],
                          start=st, stop=sp, tile_position=(0, 0))
                      nc.tensor.matmul(
                          ps3b[:],
                          cb[64:112, CB_W3 + k * 64:CB_W3 + k * 64 + 64],
                          c2sb[64:112, :, dy:dy + 15:2, dx:dx + 15:2],
                          start=st, stop=sp, tile_position=(64, 0))
                  nc.scalar.activation(
                      featc[0:64, 0:192].rearrange("p (i m) -> p i m", m=M),
                      ps3a[:].rearrange("p i a b -> p i (a b)"),
                      AF.Relu, bias=bc3)
                  nc.vector.tensor_scalar(
                      featc[0:64, 192:384].rearrange("p (i m) -> p i m", m=M),
                      ps3b[:].rearrange("p i a b -> p i (a b)"),
                      bc3, 0.0, op0=OP.add, op1=OP.max)

                # ---- cls: per-image channel sums, rest on host.
                # The DVE reduce is emitted AFTER the hdd stream (DVE is the
                # relation bottleneck; in the tail it has idle cycles). ----
                fme = wpool.tile([64, S], F32)
                nc.gpsimd.memset(fme[:], 0.0)

                # ---- u / v ----
                u_f32 = wpool.tile([H1, S * M], F32)
                v_bf = wpool.tile([H1, S * M], BF16)
                if _do("uv"):
                    psu = ps_pool.tile([H1, S * M], F32, tag="sm")
                    psv = ps_pool.tile([H1, S * M], F32, tag="sm")
                    # v first: it gates every hdd op; u only gates per-column
                    nc.tensor.matmul(psv[:], w1b, featc[:], start=True, stop=True)
                    nc.tensor.matmul(psu[:], w1a, featc[:], start=True, stop=True)
                    # v on ACT, u on DVE: the two copies run concurrently
                    nc.scalar.activation(v_bf[:], psv[:], AF.Identity, bias=bg1)
                    nc.vector.tensor_copy(u_f32[:], psu[:])
                else:
                    nc.gpsimd.memset(u_f32[:], 0.0)
                    nc.gpsimd.memset(v_bf[:], 0.0)

            # ---- relation stage ----
            KH_ACT0 = int(os.environ.get("KH_ACT0", "12"))
            with tc.tile_pool(name="pbig", bufs=2, space="PSUM") as pb_pool:
              if _do("rel"):
                  # pre-allocate and manually rotate tiles: pool-per-iteration
                  # tiles each cost a TileRelease w/ cross-engine sync at exit
                  NHB = int(os.environ.get("KHB", "3"))
                  hdd_t = [hpool.tile([H1, 32, S * M], BF16, tag="hdd",
                                      name=f"hddt{i}") for i in range(NHB)]
                  ps_t = [pb_pool.tile([128, 2048], F32, tag="gps",
                                       name=f"pst{i}") for i in range(2)]
                  gscr_t = [spool.tile([128, 2048], BF16, tag="gscr",
                                       name=f"gscrt{i}") for i in range(3)]
                  psc = [0, 0]
                  # featc/u/v hold images in slot order [0, 2, 4, 1, 3, 5];
                  # local j-images 0,1,2 live in slots 0, 3, 1.
                  for jl, jsl in enumerate((0, 3, 1)):
                      for qh in range(2):
                          unit = jl * 2 + qh
                          hdd = hdd_t[unit % NHB]
                          # unit 5 keeps its hdd fully on DVE: ACT is gscr-
                          # backlogged at that point and would gate the final
                          # PE matmuls later than DVE does
                          n_act = (KH_ACT0 if unit == 0 else
                                   int(os.environ.get("KH_LAST", "1"))
                                   if unit == 5 else KH_ACT)
                          hdd_act = set(range(32 - n_act, 32))
                          for ql in range(32):
                              q = qh * 32 + ql
                              ucol = u_f32[:, jsl * M + q: jsl * M + q + 1]
                              if ql in hdd_act:
                                  nc.scalar.activation(hdd[:, ql, :], v_bf[:],
                                                       AF.Relu, bias=ucol)
                              else:
                                  nc.vector.tensor_scalar(hdd[:, ql, :], v_bf[:],
                                                          ucol, 0.0,
                                                          op0=OP.add, op1=OP.max)
                          if unit == 5 and _do("uv"):
                              # DVE idles briefly right after its final hdd op
                              # (PE still filling psum): slot the cls reduce in
                              nc.vector.tensor_reduce(
                                  fme[:],
                                  featc[0:64, :].rearrange(
                                      "p (i m) -> p i m", m=M),
                                  axis=mybir.AxisListType.X, op=OP.add)
                              nc.sync.dma_start(out=out_fme[:], in_=fme[:])
                          if unit == 5 and os.environ.get("KU5", "1") == "1":
                              # tail unit: 4-deep rotation of 2-bank psum
                              # half-slots with ACT || DVE alternating drains
                              # -> finer fill/drain overlap while both
                              # engines wind down
                              for duo in range(3):
                                  iA, iB = 2 * duo, 2 * duo + 1
                                  col = unit * 3 + duo
                                  for hf in range(2):
                                      t = psc[0] % 4
                                      psh = ps_t[t // 2][
                                          :, (t % 2) * 1024:(t % 2) * 1024 + 1024]
                                      psc[0] += 1
                                      for qg2 in range(2):
                                          qg = hf * 2 + qg2
                                          nc.tensor.matmul(
                                              psh[0:CO, qg2 * 512:(qg2 + 1) * 512],
                                              wg2,
                                              hdd[:, qg * 8:(qg + 1) * 8,
                                                  iA * M:(iA + 1) * M],
                                              start=True, stop=True,
                                              tile_position=(0, 0))
                                          nc.tensor.matmul(
                                              psh[CO:2 * CO,
                                                  qg2 * 512:(qg2 + 1) * 512],
                                              wg2,
                                              hdd[:, qg * 8:(qg + 1) * 8,
                                                  iB * M:(iB + 1) * M],
                                              start=True, stop=True,
                                              tile_position=(0, 64))
                                      gscr = gscr_t[psc[1] % 3][:, 0:1024]
                                      psc[1] += 1
                                      if hf == 0:
                                          nc.scalar.activation(
                                              gscr[:], psh[:], AF.Relu,
                                              bias=bg2,
                                              accum_out=xf_a[:, col:col + 1])
                                      else:
                                          nc.vector.scalar_tensor_tensor(
                                              gscr[:], psh[:], bg2,
                                              zb2048[:, 0:1024],
                                              op0=OP.add, op1=OP.max,
                                              accum_out=xf_d[:, col:col + 1])
                              continue
                          if unit == 0 and os.environ.get("KU0", "0") == "1":
                              for duo in range(3):
                                  iA, iB = 2 * duo, 2 * duo + 1
                                  col = unit * 3 + duo
                                  for hf in range(2):
                                      psh = ps_t[psc[0] % 2][:, 0:1024]
                                      psc[0] += 1
                                      for qg2 in range(2):
                                          qg = hf * 2 + qg2
                                          nc.tensor.matmul(
                                              psh[0:CO, qg2 * 512:(qg2 + 1) * 512],
                                              wg2,
                                              hdd[:, qg * 8:(qg + 1) * 8,
                                                  iA * M:(iA + 1) * M],
                                              start=True, stop=True,
                                              tile_position=(0, 0))
                                          nc.tensor.matmul(
                                              psh[CO:2 * CO, qg2 * 512:(qg2 + 1) * 512],
                                              wg2,
                                              hdd[:, qg * 8:(qg + 1) * 8,
                                                  iB * M:(iB + 1) * M],
                                              start=True, stop=True,
                                              tile_position=(0, 64))
                                      gscr = gscr_t[psc[1] % 3][:, 0:1024]
                                      psc[1] += 1
                                      xft = xf_a if hf == 0 else xf_d
                                      nc.scalar.activation(
                                          gscr[:], psh[:], AF.Relu, bias=bg2,
                                          accum_out=xft[:, col:col + 1])
                              continue
                          for duo in range(3):
                              iA, iB = 2 * duo, 2 * duo + 1
                              ps = ps_t[psc[0] % 2][:]
                              psc[0] += 1
                              for qg in range(4):
                                  nc.tensor.matmul(
                                      ps[0:CO, qg * 512:(qg + 1) * 512],
                                      wg2,
                                      hdd[:, qg * 8:(qg + 1) * 8, iA * M:(iA + 1) * M],
                                      start=True, stop=True,
                                      tile_position=(0, 0))
                                  nc.tensor.matmul(
                                      ps[CO:2 * CO, qg * 512:(qg + 1) * 512],
                                      wg2,
                                      hdd[:, qg * 8:(qg + 1) * 8, iB * M:(iB + 1) * M],
                                      start=True, stop=True,
                                      tile_position=(0, 64))
                              col = unit * 3 + duo
                              gscr = gscr_t[psc[1] % 3][:]
                              psc[1] += 1
                              if KSPLIT and unit == 5:
                                  # final tile: drain both halves concurrently
                                  nc.scalar.activation(
                                      gscr[:, 0:1024], ps[:, 0:1024],
                                      AF.Relu, bias=bg2,
                                      accum_out=xf_a[:, col:col + 1])
                                  nc.vector.scalar_tensor_tensor(
                                      gscr[:, 1024:2048], ps[:, 1024:2048],
                                      bg2, zb2048[:, 0:1024],
                                      op0=OP.add, op1=OP.max,
                                      accum_out=xf_d[:, col:col + 1])
                              elif duo < KG_V or (unit, duo) in KDVE:
                                  nc.vector.scalar_tensor_tensor(
                                      gscr[:], ps[:], bg2, zb2048[:],
                                      op0=OP.add, op1=OP.max,
                                      accum_out=xf_d[:, col:col + 1])
                              else:
                                  nc.scalar.activation(
                                      gscr[:], ps[:], AF.Relu, bias=bg2,
                                      accum_out=xf_a[:, col:col + 1])

            if not _do("uv"):
                nc.sync.dma_start(out=out_fme[:], in_=fme[:])
            nc.sync.dma_start(out=out_xf[:], in_=xf_all[:])
    nc.compile()
    return nc


_NC_CACHE = None


def _get_nc():
    global _NC_CACHE
    if _NC_CACHE is None:
        _NC_CACHE = _build_nc()
    return _NC_CACHE


def _host_prep(inputs):
    ins = {k: np.asarray(v) for k, v in inputs.items()}
    x = np.concatenate([ins['support_x'], ins['query_x']], axis=1)
    lab = np.concatenate([ins['support_y'], ins['query_y']], axis=1)

    xpad = np.pad(x.astype(np.float32), ((0, 0), (0, 0), (0, 0), (0, 1), (0, 1)))
    win = np.lib.stride_tricks.sliding_window_view(xpad, (3, 3), axis=(3, 4))
    win = win[:, :, :, ::2, ::2]
    patches = win.transpose(0, 2, 5, 6, 1, 3, 4).reshape(B, 27, S, 1024)
    patches = np.ascontiguousarray(patches, np.float32)

    f32 = np.float32
    bf16 = ml_dtypes.bfloat16

    w1 = ins['k1'].reshape(32, 27).T
    w2 = ins['k2'].transpose(1, 2, 3, 0).reshape(32, 432)
    w3 = ins['k3'].transpose(1, 2, 3, 0).reshape(48, 576)
    cb = np.zeros((128, CB_N), f32)
    for g in range(4):
        cb[32 * g:32 * g + 27, CB_W1:CB_W1 + 32] = w1
        cb[32 * g:32 * g + 32, CB_W2:CB_W2 + 432] = w2
    cb[0:48, CB_W3:CB_W3 + 576] = w3
    cb[64:112, CB_W3:CB_W3 + 576] = w3
    Wg1 = ins['Wg1'].astype(f32)
    cb[0:C2, CB_W1A:CB_W1A + H1] = Wg1[:C2]
    cb[0:C2, CB_W1B:CB_W1B + H1] = Wg1[C2:]
    cb[:, CB_WG2:CB_WG2 + CO] = ins['Wg2']
    ii = np.arange(D, dtype=f32) / D
    coord = np.stack([np.broadcast_to(ii[:, None], (D, D)),
                      np.broadcast_to(ii[None, :], (D, D))]).reshape(2, M)
    cb[0:2, CB_COORD:CB_COORD + 384] = np.tile(coord, (1, S))
    cb = cb.astype(bf16)

    cf = np.zeros((128, CF_N), f32)
    for g in range(4):
        cf[32 * g:32 * g + 32, 0] = ins['bc1']
    cf[0:48, 1] = ins['bc2']
    cf[64:112, 1] = ins['bc2']
    cf[0:64, 2] = ins['bc3']
    cf[:, 3] = ins['bg1']
    cf[:, 4] = np.tile(ins['bg2'].astype(f32), 2)

    in_maps = []
    for core in range(N_CORES):
        b, half = core // 2, core % 2
        perm = (0, 1, 2, 3, 4, 5) if half == 0 else (3, 4, 5, 0, 1, 2)
        pc = patches[b][:, perm, :]              # [27, S, 1024]
        pk = np.zeros((128, 4, 512), f32)
        for blk in range(4):
            h, d = blk % 2, blk // 2
            for g in range(4 if blk < 2 else 2):
                img = g + 4 * d
                pk[32 * g:32 * g + 27, blk, :] = pc[:, img, h * 512:(h + 1) * 512]
        m = dict(cb=cb, cf=cf, pk=pk.astype(bf16))
        in_maps.append(m)
    return in_maps, lab, ins


def _host_post(results, lab, ins):
    f64 = np.float64
    Wf1 = ins['Wf1'].astype(f64)
    bf1 = ins['bf1'].astype(f64)
    Wf2 = ins['Wf2'].astype(f64)
    bf2 = ins['bf2'].astype(f64)
    Wlog = ins['Wlog'].astype(f64)
    blog = ins['blog'].astype(f64)

    P = np.zeros((B, S, S), f64)
    cls_terms = np.zeros((B, S), f64)
    for core in range(N_CORES):
        b, half = core // 2, core % 2
        perm = (0, 1, 2, 3, 4, 5) if half == 0 else (3, 4, 5, 0, 1, 2)
        slot_img = (0, 2, 4, 1, 3, 5)                   # image held by slot s
        xfp = results[core]["xf"].astype(f64)           # [128, 38] packed
        xf = xfp[:, 0:18] + xfp[:, 20:38]               # [128, 18]
        xf9 = xf.reshape(128, 3, 2, 3).sum(axis=2)      # (jl, duo)
        for jl in range(3):
            for duo in range(3):
                for par in range(2):
                    i = slot_img[2 * duo + par]
                    vec = xf9[par * 64:(par + 1) * 64, jl, duo]
                    h = np.maximum(vec @ Wf1 + bf1, 0.0)
                    z = h @ Wf2 + bf2
                    P[b, perm[i], perm[jl]] = 1.0 / (1.0 + np.exp(-z[0]))
        if half == 0:
            fme = results[core]["fme"].astype(f64)      # [64, S] channel sums
            logits = (fme.T / M) @ Wlog + blog          # rows = slots
            mx = logits.max(axis=1)
            lse = mx + np.log(np.exp(logits - mx[:, None]).sum(axis=1))
            for s in range(S):
                img = slot_img[s]
                cls_terms[b, img] = lse[s] - logits[s, lab[b][img]]

    cls_loss = cls_terms.mean()
    y = (lab[:, :, None] == lab[:, None, :]).astype(f64)
    Pt = P.transpose(0, 2, 1)
    sym, anti = 0.5 * (P + Pt), 0.5 * (P - Pt)
    sym_n = np.sqrt((sym ** 2).sum(axis=(1, 2)))
    anti_n = np.sqrt((anti ** 2).sum(axis=(1, 2)))
    sym_loss = ((sym_n - anti_n) / (sym_n + anti_n)).mean()
    euc_loss = ((P - y) ** 2).mean()
    rn_loss = euc_loss - 0.1 * sym_loss
    return np.float32(cls_loss), np.float32(rn_loss), np.float32(sym_loss)


def run_spmd(inputs, trace=False, **kwargs):
    nc = _get_nc()
    in_maps, lab, ins = _host_prep(inputs)
    res = run_bass_kernel_spmd(nc, in_maps, list(range(N_CORES)),
                               trace=trace, **kwargs)
    return _host_post(res.results, lab, ins), res


def kernel(**inputs):
    out, _ = run_spmd(inputs)
    return out

